# revision 5
# baseline (speedup 1.0000x reference)
"""Trainium2 Bass kernel for the brute-force antisymmetrized ResNet.

Math (per walker b):
    feats[i,j] = concat(x1[P1[i]], x2[P2[j]]).reshape(24)    (576 = 24*24 perm pairs)
    y0 = tanh(feats @ W0 + b0)
    y1 = tanh(y0 @ W1 + b1) + y0
    y2 = tanh(y1 @ W2 + b2) + y1
    out[b] = log| sum_{i,j} s1[i] s2[j] (y2[i,j] @ Wf + bf) |

Strategy (v2, instruction-count-driven):
  - The dominant runtime cost on this stack is per-dynamic-instruction
    overhead (~tens of us regardless of op size), so the kernel is
    restructured around hardware loops (tc.For_i) and few, maximally
    fat instructions.
  - Data-parallel over walkers: 64 walkers x 8 cores.  Rows ordered
    (walker, i, j) in natural perm order; signs applied on host.
  - First layer factored: h0pre[w,i,j] = u1[w,i] + u2[w,j], built per
    16-walker group with ONE broadcast-add per half + ONE tanh.
  - Main loop: 1024-row tiles, 9 per group, as a hardware loop with
    register-offset APs.  Per tile: 8 fp32 matmuls (L1) + 1 tanh + 1
    residual add + 8 matmuls (L2) + 1 tanh + 1 add + 4 head matmuls
    (y2 @ Wf into PSUM partition 0, reusing psB's bank space after
    tanh2 drains it) + 1 copy of the 1024 head values to a staging row.
  - Device returns ONLY the per-row head scalars f[w,i,j] (36864 fp32
    per core).  Host applies the sign pattern and sums in fp64 - this
    removes the catastrophic 288-row fp32 segment-sum rounding of the
    previous design (rel err 2.6e-2 -> ~1e-3) AND all on-device
    reduction instructions.
"""

import itertools

import numpy as np

N1 = 4
N2 = 4
D = 3
BATCH = 512
NDENSE = 256
NCORES = 8
NPERM = 24               # 4!
NPAIR = NPERM * NPERM    # 576
WPC = BATCH // NCORES    # 64 walkers per core
ROWS = WPC * NPAIR       # 36864 rows per core
GW = 16                  # walkers per h0 group
GROUP_ROWS = GW * NPAIR  # 9216
NG = WPC // GW           # 4 groups
TILE = 1024              # rows per main-loop tile
TPG = GROUP_ROWS // TILE # 9 tiles per group
UCOLS = WPC * NPERM      # 1536 u-columns per core
K1 = N1 * D + 1          # 13: x1 features + ones row (carries b0)
K2 = N2 * D              # 12


def _perms_and_signs(n):
    P = np.array(list(itertools.permutations(range(n))), dtype=np.int32)
    triu = np.triu(np.ones((n, n), dtype=np.int64), 1)
    inv = np.sum((P[:, :, None] > P[:, None, :]) * triu, axis=(1, 2))
    signs = np.where(inv % 2 == 0, 1.0, -1.0).astype(np.float32)
    return P, signs


_P1, _S1 = _perms_and_signs(N1)
_P2, _S2 = _perms_and_signs(N2)
_SGN = (_S1[:, None] * _S2[None, :]).reshape(NPAIR).astype(np.float64)

_cached = {}
_last_results = None  # BassKernelResults of the most recent run (for profiling)


def _build_nc(with_bias: bool):
    """Build + compile the 8-core SPMD Tile kernel (cached)."""
    key = bool(with_bias)
    if key in _cached:
        return _cached[key]

    import concourse.bacc as bacc
    import concourse.tile as tile
    from concourse import mybir
    from concourse.bass import ds, ts

    FP = mybir.dt.float32
    TANH = mybir.ActivationFunctionType.Tanh

    nc = bacc.Bacc(
        "TRN2",
        target_bir_lowering=False,
        debug=False,
        num_devices=NCORES,
    )

    x1f_d = nc.dram_tensor("x1f", [K1, UCOLS], FP, kind="ExternalInput").ap()
    x2f_d = nc.dram_tensor("x2f", [K2, UCOLS], FP, kind="ExternalInput").ap()
    x1w_d = nc.dram_tensor("x1w", [K1, NDENSE], FP, kind="ExternalInput").ap()
    x2w_d = nc.dram_tensor("x2w", [K2, NDENSE], FP, kind="ExternalInput").ap()
    w1_d = nc.dram_tensor("w1", [NDENSE, NDENSE], FP, kind="ExternalInput").ap()
    w2_d = nc.dram_tensor("w2", [NDENSE, NDENSE], FP, kind="ExternalInput").ap()
    wf_d = nc.dram_tensor("wf", [128, 2], FP, kind="ExternalInput").ap()
    if with_bias:
        b_d = nc.dram_tensor("b12", [128, 4], FP, kind="ExternalInput").ap()
    f_d = nc.dram_tensor("f", [1, ROWS], FP, kind="ExternalOutput").ap()

    with tile.TileContext(nc) as tc:
        with (
            tc.tile_pool(name="consts", bufs=1) as cpool,
            tc.tile_pool(name="ps", bufs=1, space="PSUM") as pspool,
        ):
            x1f = cpool.tile([K1, UCOLS], FP, tag="x1f")
            nc.sync.dma_start(x1f[:], x1f_d[:])
            x2f = cpool.tile([K2, UCOLS], FP, tag="x2f")
            nc.sync.dma_start(x2f[:], x2f_d[:])
            x1w = cpool.tile([K1, NDENSE], FP, tag="x1w")
            nc.sync.dma_start(x1w[:], x1w_d[:])
            x2w = cpool.tile([K2, NDENSE], FP, tag="x2w")
            nc.sync.dma_start(x2w[:], x2w_d[:])
            w1a = cpool.tile([128, NDENSE], FP, tag="w1a")
            nc.sync.dma_start(w1a[:], w1_d[0:128, :])
            w1b = cpool.tile([128, NDENSE], FP, tag="w1b")
            nc.sync.dma_start(w1b[:], w1_d[128:256, :])
            w2a = cpool.tile([128, NDENSE], FP, tag="w2a")
            nc.sync.dma_start(w2a[:], w2_d[0:128, :])
            w2b = cpool.tile([128, NDENSE], FP, tag="w2b")
            nc.sync.dma_start(w2b[:], w2_d[128:256, :])
            wf = cpool.tile([128, 2], FP, tag="wf")
            nc.sync.dma_start(wf[:], wf_d[:])
            if with_bias:
                bsb = cpool.tile([128, 4], FP, tag="b12")  # b1h0 b1h1 b2h0 b2h1
                nc.sync.dma_start(bsb[:], b_d[:])

            u1s = cpool.tile([128, 2, UCOLS], FP, tag="u1s")
            u2s = cpool.tile([128, 2, UCOLS], FP, tag="u2s")
            h0 = cpool.tile([128, 2, GROUP_ROWS], FP, tag="h0")
            t1 = cpool.tile([128, 2, TILE], FP, tag="t1")
            y2 = cpool.tile([128, 2, TILE], FP, tag="y2")
            fsb = cpool.tile([1, GROUP_ROWS], FP, tag="fsb")

            # ---- u-phase: first-layer partials u1[w,i], u2[w,j] ----
            # u1s[p, h, w*24+i] = (x1 perm-feats @ W0[:12] + b0)[w, i, h*128+p]
            # Stationary-major order: weight reloads dominate matmul cost, so
            # each distinct stationary streams all its moving chunks in a row.
            for usb, xf, xw in ((u1s, x1f, x1w), (u2s, x2f, x2w)):
                ps = pspool.tile([128, 2, TILE], FP, tag="psA", name="psu0")
                ps2 = pspool.tile([128, 2, TILE], FP, tag="psB", name="psu1")
                for h in (0, 1):
                    for c in (0, 1, 2):
                        o = (ps if c < 2 else ps2)[:, h, (c % 2) * 512:(c % 2) * 512 + 512]
                        nc.tensor.matmul(
                            o,
                            xw[:, h * 128:(h + 1) * 128],
                            xf[:, c * 512:(c + 1) * 512],
                            start=True,
                            stop=True,
                        )
                nc.vector.tensor_copy(usb[:, :, 0:TILE], ps[:])
                nc.vector.tensor_copy(usb[:, :, TILE:UCOLS], ps2[:, :, 0:512])

            # ---- per group: build h0 = tanh(u1 (+) u2), then 9 tiles ----
            for g in range(NG):
                sl = slice(g * GW * NPERM, (g + 1) * GW * NPERM)
                for h in (0, 1):
                    in1 = u1s[:, h, sl].rearrange(
                        "p (w i u) -> p w i u", i=NPERM, u=1
                    ).broadcast_to([128, GW, NPERM, NPERM])
                    in2 = u2s[:, h, sl].rearrange(
                        "p (w u j) -> p w u j", u=1, j=NPERM
                    ).broadcast_to([128, GW, NPERM, NPERM])
                    out = h0[:, h, :].rearrange(
                        "p (w i j) -> p w i j", i=NPERM, j=NPERM
                    )
                    nc.vector.tensor_add(out, in1, in2)
                nc.scalar.activation(h0[:], h0[:], TANH)

                with tc.For_i(0, TPG) as t:
                    # layer 1: h0 @ W1 -> psA (stationary-major: both moving
                    # chunks per weight block, PSUM k-accumulation)
                    psA = pspool.tile([128, 2, TILE], FP, tag="psA", name="psA")
                    for h in (0, 1):
                        for ka, kw in ((0, w1a), (1, w1b)):
                            for c in (0, 1):
                                nc.tensor.matmul(
                                    psA[:, h, c * 512:(c + 1) * 512],
                                    kw[:, h * 128:(h + 1) * 128],
                                    h0[:, ka, ds(t * TILE + c * 512, 512)],
                                    start=(ka == 0),
                                    stop=(ka == 1),
                                )
                    # tanh1 (+ b1) then residual add of h0 rows
                    if with_bias:
                        for h in (0, 1):
                            nc.scalar.activation(
                                t1[:, h, :], psA[:, h, :], TANH,
                                bias=bsb[:, h:h + 1],
                            )
                    else:
                        nc.scalar.activation(t1[:], psA[:], TANH)
                    nc.vector.tensor_add(
                        t1[:], t1[:],
                        h0[:, :, ts(t, TILE)],
                    )
                    # layer 2: t1 @ W2 -> psB
                    psB = pspool.tile([128, 2, TILE], FP, tag="psB", name="psB")
                    for h in (0, 1):
                        for ka, kw in ((0, w2a), (1, w2b)):
                            for c in (0, 1):
                                nc.tensor.matmul(
                                    psB[:, h, c * 512:(c + 1) * 512],
                                    kw[:, h * 128:(h + 1) * 128],
                                    t1[:, ka, c * 512:(c + 1) * 512],
                                    start=(ka == 0),
                                    stop=(ka == 1),
                                )
                    # t2 = tanh2 (+ b2); the y2 = t2 + t1 residual is folded
                    # into the head: f = t1 @ Wf + t2 @ Wf, accumulated
                    # exactly in PSUM partition 0 of psB (banks free once
                    # tanh2 has drained them)
                    if with_bias:
                        for h in (0, 1):
                            nc.scalar.activation(
                                y2[:, h, :], psB[:, h, :], TANH,
                                bias=bsb[:, 2 + h:3 + h],
                            )
                    else:
                        nc.scalar.activation(y2[:], psB[:], TANH)
                    psf = psB[0:1, 0, :]
                    for i, (h, src) in enumerate(
                        ((0, t1), (0, y2), (1, t1), (1, y2))
                    ):
                        for c in (0, 1):
                            nc.tensor.matmul(
                                psf[:, c * 512:(c + 1) * 512],
                                wf[:, h:h + 1],
                                src[:, h, c * 512:(c + 1) * 512],
                                start=(i == 0),
                                stop=(i == 3),
                            )
                    nc.vector.tensor_copy(
                        fsb[0:1, ts(t, TILE)], psf
                    )

                nc.sync.dma_start(
                    f_d[0:1, g * GROUP_ROWS:(g + 1) * GROUP_ROWS], fsb[0:1, :]
                )

    nc.compile()
    _cached[key] = nc
    return nc


def _build_feats(x1, x2):
    """Per-walker first-layer inputs, natural lexicographic perm order.

    Returns (X1f [B, 24, 13], X2f [B, 24, 12]): X1f[b, i] = flattened
    x1[b, P1[i]] + trailing 1.0 (carries b0); X2f likewise, no ones.
    """
    B = x1.shape[0]
    xp1 = x1[:, _P1, :].reshape(B, NPERM, N1 * D)
    xp2 = x2[:, _P2, :].reshape(B, NPERM, N2 * D)
    X1f = np.empty((B, NPERM, K1), dtype=np.float32)
    X1f[:, :, :N1 * D] = xp1
    X1f[:, :, N1 * D] = 1.0
    return X1f, np.ascontiguousarray(xp2)


def _make_in_maps(x1, x2, W0, b0, W1, b1, W2, b2, Wf):
    with_bias = bool(np.any(b1) or np.any(b2))
    X1f, X2f = _build_feats(x1, x2)
    x1w = np.ascontiguousarray(
        np.concatenate([W0[:N1 * D], b0[None, :]], axis=0)
    )  # [13, 256]
    x2w = np.ascontiguousarray(W0[N1 * D:])  # [12, 256]
    wf = np.ascontiguousarray(Wf[:, 0].reshape(2, 128).T)  # [128, 2]
    in_maps = []
    for c in range(NCORES):
        sl = slice(c * WPC, (c + 1) * WPC)
        m = {
            "x1f": np.ascontiguousarray(X1f[sl].reshape(UCOLS, K1).T),
            "x2f": np.ascontiguousarray(X2f[sl].reshape(UCOLS, K2).T),
            "x1w": x1w,
            "x2w": x2w,
            "w1": np.ascontiguousarray(W1),
            "w2": np.ascontiguousarray(W2),
            "wf": wf,
        }
        if with_bias:
            bm = np.zeros((128, 4), dtype=np.float32)
            bm[:, 0] = b1[0:128]
            bm[:, 1] = b1[128:256]
            bm[:, 2] = b2[0:128]
            bm[:, 3] = b2[128:256]
            m["b12"] = bm
        in_maps.append(m)
    return with_bias, in_maps


def _finish(f_per_core):
    """per-core f rows [1, ROWS] -> log|anti| [BATCH] via fp64 signed sums.

    bf drops out exactly: the 576 pair signs sum to zero.
    """
    out = np.empty((BATCH,), dtype=np.float32)
    for c in range(NCORES):
        f = f_per_core[c].reshape(WPC, NPAIR).astype(np.float64)
        anti = f @ _SGN
        out[c * WPC:(c + 1) * WPC] = np.log(np.abs(anti)).astype(np.float32)
    return out


def kernel(x1, x2, W0, b0, W1, b1, W2, b2, Wf, bf):
    from concourse.bass_utils import run_bass_kernel_spmd

    x1 = np.asarray(x1, dtype=np.float32)
    x2 = np.asarray(x2, dtype=np.float32)
    W0 = np.asarray(W0, dtype=np.float32)
    b0 = np.asarray(b0, dtype=np.float32)
    W1 = np.asarray(W1, dtype=np.float32)
    b1 = np.asarray(b1, dtype=np.float32)
    W2 = np.asarray(W2, dtype=np.float32)
    b2 = np.asarray(b2, dtype=np.float32)
    Wf = np.asarray(Wf, dtype=np.float32)
    bf = np.asarray(bf, dtype=np.float32)

    with_bias, in_maps = _make_in_maps(x1, x2, W0, b0, W1, b1, W2, b2, Wf)
    nc = _build_nc(with_bias)

    try:
        res = run_bass_kernel_spmd(nc, in_maps, list(range(NCORES)))
    except ModuleNotFoundError:
        # BASS_TRACE requested but this axon build lacks the NTFF profile
        # hook module; rerun with tracing suppressed.
        import os

        os.environ["BASS_NEVER_TRACE"] = "1"
        res = run_bass_kernel_spmd(nc, in_maps, list(range(NCORES)))
    global _last_results
    _last_results = res

    return _finish([res.results[c]["f"] for c in range(NCORES)])


# revision 10
# speedup vs baseline: 1.9906x; 1.9906x over previous
"""Trainium2 Bass kernel for the brute-force antisymmetrized ResNet.

Math (per walker b):
    feats[i,j] = concat(x1[P1[i]], x2[P2[j]]).reshape(24)    (576 = 24*24 perm pairs)
    y0 = tanh(feats @ W0 + b0)
    y1 = tanh(y0 @ W1 + b1) + y0
    y2 = tanh(y1 @ W2 + b2) + y1
    out[b] = log| sum_{i,j} s1[i] s2[j] (y2[i,j] @ Wf + bf) |

Strategy (v2, instruction-count-driven):
  - The dominant runtime cost on this stack is per-dynamic-instruction
    overhead (~tens of us regardless of op size), so the kernel is
    restructured around hardware loops (tc.For_i) and few, maximally
    fat instructions.
  - Data-parallel over walkers: 64 walkers x 8 cores.  Rows ordered
    (walker, i, j) in natural perm order; signs applied on host.
  - First layer factored: h0pre[w,i,j] = u1[w,i] + u2[w,j], built per
    16-walker group with ONE broadcast-add per half + ONE tanh.
  - Main loop: 1536-row tiles, 6 per group, as a hardware loop with
    register-offset APs.  Matmuls are stationary-major (a weight block
    streams all three 512-col moving chunks before switching - weight
    reloads dominate matmul cost here).  Per tile: 12 fp32 matmuls
    (L1, 6-bank PSUM) + tanh + residual add + 12 matmuls (L2, reusing
    the same banks once tanh1 drained them) + tanh + 12 head matmuls
    (f = t1 @ Wf + tanh2 @ Wf accumulated exactly in PSUM partition 0;
    the y2 residual add is folded into the head by linearity) + 1 copy
    of the head values to a staging row; one DMA per group.
  - Device returns ONLY the per-row head scalars f[w,i,j] (36864 fp32
    per core).  Host applies the sign pattern and sums in fp64 - this
    removes the catastrophic 288-row fp32 segment-sum rounding of the
    previous design (rel err 2.6e-2 -> ~3e-3) AND all on-device
    reduction instructions.
"""

import itertools

import numpy as np

N1 = 4
N2 = 4
D = 3
BATCH = 512
NDENSE = 256
NCORES = 8
NPERM = 24               # 4!
NPAIR = NPERM * NPERM    # 576
WPC = BATCH // NCORES    # 64 walkers per core
ROWS = WPC * NPAIR       # 36864 rows per core
GW = 16                  # walkers per h0 group
GROUP_ROWS = GW * NPAIR  # 9216
NG = WPC // GW           # 4 groups
TILE = 1536              # rows per main-loop tile (3 x 512 moving chunks)
TPG = GROUP_ROWS // TILE # 6 tiles per group
UCOLS = WPC * NPERM      # 1536 u-columns per core
K1 = N1 * D + 1          # 13: x1 features + ones row (carries b0)
K2 = N2 * D              # 12


def _perms_and_signs(n):
    P = np.array(list(itertools.permutations(range(n))), dtype=np.int32)
    triu = np.triu(np.ones((n, n), dtype=np.int64), 1)
    inv = np.sum((P[:, :, None] > P[:, None, :]) * triu, axis=(1, 2))
    signs = np.where(inv % 2 == 0, 1.0, -1.0).astype(np.float32)
    return P, signs


_P1, _S1 = _perms_and_signs(N1)
_P2, _S2 = _perms_and_signs(N2)
_SGN = (_S1[:, None] * _S2[None, :]).reshape(NPAIR).astype(np.float64)

_cached = {}
_last_results = None  # BassKernelResults of the most recent run (for profiling)


def _build_nc(with_bias: bool):
    """Build + compile the 8-core SPMD Tile kernel (cached)."""
    key = bool(with_bias)
    if key in _cached:
        return _cached[key]

    import concourse.bacc as bacc
    import concourse.tile as tile
    from concourse import mybir
    from concourse.bass import ds, ts

    FP = mybir.dt.float32
    TANH = mybir.ActivationFunctionType.Tanh

    nc = bacc.Bacc(
        "TRN2",
        target_bir_lowering=False,
        debug=False,
        num_devices=NCORES,
    )

    x1f_d = nc.dram_tensor("x1f", [K1, UCOLS], FP, kind="ExternalInput").ap()
    x2f_d = nc.dram_tensor("x2f", [K2, UCOLS], FP, kind="ExternalInput").ap()
    x1w_d = nc.dram_tensor("x1w", [K1, NDENSE], FP, kind="ExternalInput").ap()
    x2w_d = nc.dram_tensor("x2w", [K2, NDENSE], FP, kind="ExternalInput").ap()
    w1_d = nc.dram_tensor("w1", [NDENSE, NDENSE], FP, kind="ExternalInput").ap()
    w2_d = nc.dram_tensor("w2", [NDENSE, NDENSE], FP, kind="ExternalInput").ap()
    wf_d = nc.dram_tensor("wf", [128, 2], FP, kind="ExternalInput").ap()
    if with_bias:
        b_d = nc.dram_tensor("b12", [128, 4], FP, kind="ExternalInput").ap()
    f_d = nc.dram_tensor("f", [1, ROWS], FP, kind="ExternalOutput").ap()

    with tile.TileContext(nc) as tc:
        with (
            tc.tile_pool(name="consts", bufs=1) as cpool,
            tc.tile_pool(name="ps", bufs=1, space="PSUM") as pspool,
        ):
            x1f = cpool.tile([K1, UCOLS], FP, tag="x1f")
            nc.sync.dma_start(x1f[:], x1f_d[:])
            x2f = cpool.tile([K2, UCOLS], FP, tag="x2f")
            nc.sync.dma_start(x2f[:], x2f_d[:])
            x1w = cpool.tile([K1, NDENSE], FP, tag="x1w")
            nc.sync.dma_start(x1w[:], x1w_d[:])
            x2w = cpool.tile([K2, NDENSE], FP, tag="x2w")
            nc.sync.dma_start(x2w[:], x2w_d[:])
            w1a = cpool.tile([128, NDENSE], FP, tag="w1a")
            nc.sync.dma_start(w1a[:], w1_d[0:128, :])
            w1b = cpool.tile([128, NDENSE], FP, tag="w1b")
            nc.sync.dma_start(w1b[:], w1_d[128:256, :])
            w2a = cpool.tile([128, NDENSE], FP, tag="w2a")
            nc.sync.dma_start(w2a[:], w2_d[0:128, :])
            w2b = cpool.tile([128, NDENSE], FP, tag="w2b")
            nc.sync.dma_start(w2b[:], w2_d[128:256, :])
            wf = cpool.tile([128, 2], FP, tag="wf")
            nc.sync.dma_start(wf[:], wf_d[:])
            if with_bias:
                bsb = cpool.tile([128, 4], FP, tag="b12")  # b1h0 b1h1 b2h0 b2h1
                nc.sync.dma_start(bsb[:], b_d[:])

            u1s = cpool.tile([128, 2, UCOLS], FP, tag="u1s")
            u2s = cpool.tile([128, 2, UCOLS], FP, tag="u2s")
            h0 = cpool.tile([128, 2, GROUP_ROWS], FP, tag="h0")
            t1 = cpool.tile([128, 2, TILE], FP, tag="t1")
            t2 = cpool.tile([128, 2, TILE], FP, tag="t2")
            fsb = cpool.tile([1, GROUP_ROWS], FP, tag="fsb")

            # ---- u-phase: first-layer partials u1[w,i], u2[w,j] ----
            # u1s[p, h, w*24+i] = (x1 perm-feats @ W0[:12] + b0)[w, i, h*128+p]
            # Stationary-major order: weight reloads dominate matmul cost, so
            # each distinct stationary streams all its moving chunks in a row.
            for usb, xf, xw in ((u1s, x1f, x1w), (u2s, x2f, x2w)):
                ps = pspool.tile([128, 2, TILE], FP, tag="psA", name="psu0")
                for h in (0, 1):
                    for c in (0, 1, 2):
                        nc.tensor.matmul(
                            ps[:, h, c * 512:(c + 1) * 512],
                            xw[:, h * 128:(h + 1) * 128],
                            xf[:, c * 512:(c + 1) * 512],
                            start=True,
                            stop=True,
                        )
                nc.vector.tensor_copy(usb[:], ps[:])

            # ---- per group: build h0 = tanh(u1 (+) u2), then 9 tiles ----
            for g in range(NG):
                sl = slice(g * GW * NPERM, (g + 1) * GW * NPERM)
                for h in (0, 1):
                    in1 = u1s[:, h, sl].rearrange(
                        "p (w i u) -> p w i u", i=NPERM, u=1
                    ).broadcast_to([128, GW, NPERM, NPERM])
                    in2 = u2s[:, h, sl].rearrange(
                        "p (w u j) -> p w u j", u=1, j=NPERM
                    ).broadcast_to([128, GW, NPERM, NPERM])
                    out = h0[:, h, :].rearrange(
                        "p (w i j) -> p w i j", i=NPERM, j=NPERM
                    )
                    nc.vector.tensor_add(out, in1, in2)
                nc.scalar.activation(h0[:], h0[:], TANH)

                with tc.For_i(0, TPG) as t:
                    # layer 1: h0 @ W1 -> psA (stationary-major: all three
                    # moving chunks stream per weight block, PSUM k-accum)
                    psA = pspool.tile([128, 2, TILE], FP, tag="psA", name="psA")
                    for h in (0, 1):
                        for ka, kw in ((0, w1a), (1, w1b)):
                            for c in (0, 1, 2):
                                nc.tensor.matmul(
                                    psA[:, h, c * 512:(c + 1) * 512],
                                    kw[:, h * 128:(h + 1) * 128],
                                    h0[:, ka, ds(t * TILE + c * 512, 512)],
                                    start=(ka == 0),
                                    stop=(ka == 1),
                                )
                    # tanh1 (+ b1) then residual add of h0 rows
                    if with_bias:
                        for h in (0, 1):
                            nc.scalar.activation(
                                t1[:, h, :], psA[:, h, :], TANH,
                                bias=bsb[:, h:h + 1],
                            )
                    else:
                        nc.scalar.activation(t1[:], psA[:], TANH)
                    nc.vector.tensor_add(
                        t1[:], t1[:],
                        h0[:, :, ts(t, TILE)],
                    )
                    # layer 2: t1 @ W2 -> the same PSUM banks (psA is free
                    # once tanh1 has drained it)
                    psB = pspool.tile([128, 2, TILE], FP, tag="psA", name="psB")
                    for h in (0, 1):
                        for ka, kw in ((0, w2a), (1, w2b)):
                            for c in (0, 1, 2):
                                nc.tensor.matmul(
                                    psB[:, h, c * 512:(c + 1) * 512],
                                    kw[:, h * 128:(h + 1) * 128],
                                    t1[:, ka, c * 512:(c + 1) * 512],
                                    start=(ka == 0),
                                    stop=(ka == 1),
                                )
                    # t2 = tanh2 (+ b2); the y2 = t2 + t1 residual is folded
                    # into the head: f = t1 @ Wf + t2 @ Wf, accumulated
                    # exactly in PSUM partition 0 of psB (banks free once
                    # tanh2 has drained them)
                    if with_bias:
                        for h in (0, 1):
                            nc.scalar.activation(
                                t2[:, h, :], psB[:, h, :], TANH,
                                bias=bsb[:, 2 + h:3 + h],
                            )
                    else:
                        nc.scalar.activation(t2[:], psB[:], TANH)
                    psf = psB[0:1, 0, :]
                    for i, (h, src) in enumerate(
                        ((0, t1), (0, t2), (1, t1), (1, t2))
                    ):
                        for c in (0, 1, 2):
                            nc.tensor.matmul(
                                psf[:, c * 512:(c + 1) * 512],
                                wf[:, h:h + 1],
                                src[:, h, c * 512:(c + 1) * 512],
                                start=(i == 0),
                                stop=(i == 3),
                            )
                    nc.vector.tensor_copy(
                        fsb[0:1, ts(t, TILE)], psf
                    )

                nc.sync.dma_start(
                    f_d[0:1, g * GROUP_ROWS:(g + 1) * GROUP_ROWS], fsb[0:1, :]
                )

    nc.compile()
    _cached[key] = nc
    return nc


def _build_feats(x1, x2):
    """Per-walker first-layer inputs, natural lexicographic perm order.

    Returns (X1f [B, 24, 13], X2f [B, 24, 12]): X1f[b, i] = flattened
    x1[b, P1[i]] + trailing 1.0 (carries b0); X2f likewise, no ones.
    """
    B = x1.shape[0]
    xp1 = x1[:, _P1, :].reshape(B, NPERM, N1 * D)
    xp2 = x2[:, _P2, :].reshape(B, NPERM, N2 * D)
    X1f = np.empty((B, NPERM, K1), dtype=np.float32)
    X1f[:, :, :N1 * D] = xp1
    X1f[:, :, N1 * D] = 1.0
    return X1f, np.ascontiguousarray(xp2)


def _make_in_maps(x1, x2, W0, b0, W1, b1, W2, b2, Wf):
    with_bias = bool(np.any(b1) or np.any(b2))
    X1f, X2f = _build_feats(x1, x2)
    x1w = np.ascontiguousarray(
        np.concatenate([W0[:N1 * D], b0[None, :]], axis=0)
    )  # [13, 256]
    x2w = np.ascontiguousarray(W0[N1 * D:])  # [12, 256]
    wf = np.ascontiguousarray(Wf[:, 0].reshape(2, 128).T)  # [128, 2]
    in_maps = []
    for c in range(NCORES):
        sl = slice(c * WPC, (c + 1) * WPC)
        m = {
            "x1f": np.ascontiguousarray(X1f[sl].reshape(UCOLS, K1).T),
            "x2f": np.ascontiguousarray(X2f[sl].reshape(UCOLS, K2).T),
            "x1w": x1w,
            "x2w": x2w,
            "w1": np.ascontiguousarray(W1),
            "w2": np.ascontiguousarray(W2),
            "wf": wf,
        }
        if with_bias:
            bm = np.zeros((128, 4), dtype=np.float32)
            bm[:, 0] = b1[0:128]
            bm[:, 1] = b1[128:256]
            bm[:, 2] = b2[0:128]
            bm[:, 3] = b2[128:256]
            m["b12"] = bm
        in_maps.append(m)
    return with_bias, in_maps


def _finish(f_per_core):
    """per-core f rows [1, ROWS] -> log|anti| [BATCH] via fp64 signed sums.

    bf drops out exactly: the 576 pair signs sum to zero.
    """
    out = np.empty((BATCH,), dtype=np.float32)
    for c in range(NCORES):
        f = f_per_core[c].reshape(WPC, NPAIR).astype(np.float64)
        anti = f @ _SGN
        out[c * WPC:(c + 1) * WPC] = np.log(np.abs(anti)).astype(np.float32)
    return out


def kernel(x1, x2, W0, b0, W1, b1, W2, b2, Wf, bf):
    from concourse.bass_utils import run_bass_kernel_spmd

    x1 = np.asarray(x1, dtype=np.float32)
    x2 = np.asarray(x2, dtype=np.float32)
    W0 = np.asarray(W0, dtype=np.float32)
    b0 = np.asarray(b0, dtype=np.float32)
    W1 = np.asarray(W1, dtype=np.float32)
    b1 = np.asarray(b1, dtype=np.float32)
    W2 = np.asarray(W2, dtype=np.float32)
    b2 = np.asarray(b2, dtype=np.float32)
    Wf = np.asarray(Wf, dtype=np.float32)
    bf = np.asarray(bf, dtype=np.float32)

    with_bias, in_maps = _make_in_maps(x1, x2, W0, b0, W1, b1, W2, b2, Wf)
    nc = _build_nc(with_bias)

    try:
        res = run_bass_kernel_spmd(nc, in_maps, list(range(NCORES)))
    except ModuleNotFoundError:
        # BASS_TRACE requested but this axon build lacks the NTFF profile
        # hook module; rerun with tracing suppressed.
        import os

        os.environ["BASS_NEVER_TRACE"] = "1"
        res = run_bass_kernel_spmd(nc, in_maps, list(range(NCORES)))
    global _last_results
    _last_results = res

    return _finish([res.results[c]["f"] for c in range(NCORES)])
